# revision 12
# baseline (speedup 1.0000x reference)
"""LocalCorrelation (13x13 cost volume) Trainium2 kernel.

Full inputs z_t, z_t1: [8, 256, 128, 128] f32 -> out [8, 169, 128, 128] f32.
out[b, 13*di+dj, h, w] = sum_c z_t[b,c,h,w] * pad(z_t1)[b,c,h+di,w+dj] / 16

Sharding: data-parallel over batch, 1 batch element per NeuronCore (8 cores).

Per-core algorithm (SPMD, identical program):
  - Slab-staged input load (f32->bf16 SWDGE cast) interleaved with compute;
    1/sqrt(C) scale fused into the z_t reorder copy; z_t stationary tiles
    streamed per-stripe (rolling pool) to keep SBUF headroom.
  - Per 8x16 output-pixel block: TensorE "block gram" matmuls, stationary =
    z_t block [c,128 pixels], streaming = padded z_t1 20x28 window -> PSUM
    f32 (accumulated over 2 c-chunks of 128).
  - PSUM -> SBUF bf16 with 32-elem window-row pitch.
  - Sheared scratch write: DRAM addr = w*S_W + h*S_H + (p - dh + 7)*32 + q.
    The per-pixel vertical shear is absorbed into the write AP's dh
    partition stride (S_H - 32), so window row p = dh + di for every pixel
    lands at slot di+7.
  - Band gather: per (h-half, wb) one 3-dim DMA reads slots [7,20) of every
    pixel -- 832B contiguous runs, all 13 tap rows at once. h-half 0 taps
    are interleaved with stripes 8..15 (SBUF freed by streaming z_t).
  - DVE de-shear: slot pick + horizontal diagonal q = (w mod 16) + dj via
    strided copy into bf16 per-di output tiles.
  - Output write: SWDGE (GpSimd) DMA per (h-half, di) with bf16->f32 cast.
"""

import numpy as np

C = 256
H = W = 128
KS = 13
KK = 169
RAD = 6
HP = WP = 140  # padded spatial
SA = 8  # block rows (stripe height)
SB = 16  # block cols
NWB = W // SB  # 8 w-blocks per stripe
NST = H // SA  # 16 stripes
WINP = SA + 2 * RAD  # 20 streamed rows per window
WINQ = SB + 2 * RAD  # 28 streamed cols per window
QP = WINQ  # window-row pitch in xb/scratch (28)
PQ = WINP * QP  # 640 elems per pixel in xb
SCR_H = 27 * QP  # 864: scratch stride per h (27 slots: p-dh+7 in [0,27))
SCR_W = 64 * SCR_H  # 55296: scratch stride per w (per h-half)
HH = H // 2  # 64 rows per h-half
BAND = KS * QP  # 416: gathered band elems per pixel (slots 7..19)

_cache = {}


def _build():
    import concourse.bass as bass
    import concourse.mybir as mybir
    import concourse.tile as tile
    from concourse import bacc

    f32 = mybir.dt.float32
    bf16 = mybir.dt.bfloat16

    nc = bacc.Bacc("TRN2", target_bir_lowering=False, debug=False)
    zt_d = nc.dram_tensor("z_t", [C, H, W], f32, kind="ExternalInput")
    z1_d = nc.dram_tensor("z_t1", [C, H, W], f32, kind="ExternalInput")
    out_d = nc.dram_tensor("out", [KK, H, W], f32, kind="ExternalOutput")

    def alt(i):
        return nc.sync if i % 2 == 0 else nc.scalar

    with tile.TileContext(nc) as tc:
        with (
            tc.tile_pool(name="scrp", bufs=1, space="DRAM") as scrp,
            tc.tile_pool(name="persist", bufs=1) as pp,
            tc.tile_pool(name="ld", bufs=2) as ldp,
            tc.tile_pool(name="zts", bufs=2) as ztsp,
            tc.tile_pool(name="xbp", bufs=2) as xbp,
            tc.tile_pool(name="psp", bufs=2, space="PSUM") as psp,
            tc.tile_pool(name="bnd", bufs=2) as bndp,
            tc.tile_pool(name="o5p", bufs=1) as o5p,
        ):
            scr_t = [scrp.tile([W, SCR_W], bf16, tag=f"scr{i}", name=f"scr{i}")
                     for i in range(2)]
            Z1P = [pp.tile([128, HP * WP], bf16, tag=f"z1p{k}", name=f"z1p{k}")
                   for k in range(2)]
            o5 = [o5p.tile([H, KS * W], bf16, tag=f"o5_{di}", name=f"o5_{di}")
                  for di in range(KS)]
            for k in range(2):
                nc.vector.memset(Z1P[k][:, :], 0.0)

            zt_tiles = {}  # si -> [tile_k0, tile_k1] (slab of 4 stripes)

            def emit_loads(s):
                """Load 32-row slab s of z_t1 and z_t (both c-chunks)."""
                for k in range(2):
                    z1u = ldp.tile([128, 32 * W], bf16, tag="stg", name="z1u")
                    src = z1_d.ap()[k * 128:(k + 1) * 128, s * 32:(s + 1) * 32, :]
                    nc.gpsimd.dma_start(
                        z1u.rearrange("c (h w) -> c h w", h=32), src)
                    dst = Z1P[k].rearrange("c (h w) -> c h w", h=HP)[
                        :, RAD + s * 32: RAD + (s + 1) * 32, RAD: RAD + W]
                    nc.vector.tensor_copy(dst, z1u.rearrange("c (h w) -> c h w", h=32))
                for k in range(2):
                    ztu = ldp.tile([128, 32 * W], bf16, tag="stg", name="ztu")
                    src = zt_d.ap()[k * 128:(k + 1) * 128, s * 32:(s + 1) * 32, :]
                    nc.gpsimd.dma_start(
                        ztu.rearrange("c (h w) -> c h w", h=32), src)
                    # reorder to block-major with fused 1/16 scale
                    zts = ztsp.tile([128, 4 * 1024], bf16, tag=f"zts{k}",
                                    name=f"zts{k}")
                    for sl in range(4):
                        zt_tiles[(s * 4 + sl, k)] = zts
                        srcv = ztu.rearrange(
                            "c (h wb dw) -> c wb h dw", h=32, wb=NWB)[
                            :, :, sl * SA:(sl + 1) * SA, :]
                        dstv = zts[:, sl * 1024:(sl + 1) * 1024].rearrange(
                            "c (wb dh dw) -> c wb dh dw", wb=NWB, dh=SA)
                        nc.vector.tensor_scalar_mul(dstv, srcv, 1.0 / 16.0)

            def emit_stripe(si):
                hh, sil = divmod(si, 8)
                h0 = si * SA
                xb = xbp.tile([128, NWB * PQ], bf16, tag="xb", name="xb")
                for wb in range(NWB):
                    w0 = wb * SB
                    ps = [psp.tile([128, 10 * WINQ], f32, tag=f"ps{i}", name=f"ps{i}")
                          for i in range(2)]
                    for k in range(2):
                        lhsT = zt_tiles[(si, k)][
                            :, (si % 4) * 1024 + wb * 128:
                               (si % 4) * 1024 + (wb + 1) * 128]
                        for half in range(2):
                            rhs = Z1P[k].rearrange("c (h w) -> c h w", h=HP)[
                                :, h0 + 10 * half: h0 + 10 * (half + 1),
                                w0:w0 + WINQ]
                            nc.tensor.matmul(ps[half][:, :], lhsT, rhs,
                                             start=(k == 0), stop=(k == 1))
                    for half in range(2):
                        dst = xb[:, wb * PQ + half * 10 * QP:
                                 wb * PQ + (half + 1) * 10 * QP].rearrange(
                            "p (r q) -> p r q", q=QP)[:, :, :WINQ]
                        srcp = ps[half].rearrange("p (r q) -> p r q", q=WINQ)
                        if wb % 2 == 0:
                            nc.scalar.copy(dst, srcp)
                        else:
                            nc.vector.tensor_copy(dst, srcp)

                # sheared scratch write: one DMA per wb, 1280B runs.
                for wb in range(NWB):
                    off = sil * SA * SCR_H + wb * SB * SCR_W + 7 * QP
                    dst = bass.AP(scr_t[hh].tensor, off,
                                  [[SCR_H - QP, SA], [SCR_W, SB], [1, PQ]])
                    alt(wb).dma_start(dst, xb[:, wb * PQ:(wb + 1) * PQ])

            def emit_tap_unit(hh, wb):
                """Band gather + 13 de-shears for one (h-half, w-block)."""
                par = (hh * NWB + wb) % 2
                bt = bndp.tile([128, SB * BAND], bf16, tag="band", name="band")
                band = bt[par * HH:(par + 1) * HH, :]
                src = bass.AP(scr_t[hh].tensor, wb * SB * SCR_W + 7 * QP,
                              [[SCR_H, HH], [SCR_W, SB], [1, BAND]])
                alt(wb).dma_start(
                    band.rearrange("p (w e) -> p w e", e=BAND), src)
                # de-shear: o5[di][hh*64+p, dj*128 + wb*16 + wh]
                #   = band[p, wh*416 + di*32 + wh + dj]
                for di in range(KS):
                    diag = bass.AP(band.tensor, band.offset + di * QP,
                                   [list(band.ap[0]), [BAND + 1, SB], [1, KS]])
                    dst = o5[di][hh * HH:(hh + 1) * HH, :].rearrange(
                        "p (dj w) -> p dj w", dj=KS)[
                        :, :, wb * SB:(wb + 1) * SB].transpose([0, 2, 1])
                    if (wb + di) % 2 == 0:
                        nc.vector.tensor_copy(dst, diag)
                    else:
                        nc.scalar.copy(dst, diag)

            def emit_out_writes(hh):
                """SWDGE bf16->f32 cast writes, one per di."""
                for di in range(KS):
                    dstw = bass.AP(out_d, di * KS * H * W + hh * HH * W,
                                   [[W, HH], [H * W, KS], [1, W]])
                    nc.gpsimd.dma_start(
                        dstw, o5[di][hh * HH:(hh + 1) * HH, :].rearrange(
                            "p (dj w) -> p dj w", dj=KS))

            emit_loads(0)
            emit_stripe(0)
            emit_stripe(1)
            emit_loads(1)
            for si in range(2, 6):
                emit_stripe(si)
            emit_loads(2)
            for si in range(6, 10):
                emit_stripe(si)
            emit_loads(3)
            for si in range(10, 16):
                emit_stripe(si)
                emit_tap_unit(0, si - 10)
            emit_tap_unit(0, 6)
            emit_tap_unit(0, 7)
            emit_out_writes(0)
            for wb in range(NWB):
                emit_tap_unit(1, wb)
            emit_out_writes(1)

    nc.compile()
    return nc


def _get_nc():
    if "nc" not in _cache:
        _cache["nc"] = _build()
    return _cache["nc"]


def kernel(z_t: np.ndarray, z_t1: np.ndarray) -> np.ndarray:
    from concourse.bass_utils import run_bass_kernel_spmd

    nc = _get_nc()
    z_t = np.ascontiguousarray(z_t, dtype=np.float32)
    z_t1 = np.ascontiguousarray(z_t1, dtype=np.float32)
    B = z_t.shape[0]
    in_maps = [{"z_t": z_t[i], "z_t1": z_t1[i]} for i in range(B)]
    res = run_bass_kernel_spmd(nc, in_maps, core_ids=list(range(B)))
    return np.stack([res.results[i]["out"] for i in range(B)], axis=0)


# revision 13
# speedup vs baseline: 1.1252x; 1.1252x over previous
"""LocalCorrelation (13x13 cost volume) Trainium2 kernel.

Full inputs z_t, z_t1: [8, 256, 128, 128] f32 -> out [8, 169, 128, 128] f32.
out[b, 13*di+dj, h, w] = sum_c z_t[b,c,h,w] * pad(z_t1)[b,c,h+di,w+dj] / 16

Sharding: data-parallel over batch, 1 batch element per NeuronCore (8 cores).

Per-core algorithm (SPMD, identical program):
  - Slab-staged input load (f32->bf16 SWDGE cast) interleaved with compute;
    1/sqrt(C) scale fused into the z_t reorder copy; z_t stationary tiles
    streamed per-slab (rolling pool).
  - Per 8x16 output-pixel block: TensorE "block gram" matmuls, stationary =
    z_t block [c,128 pixels], streaming = padded z_t1 20x28 window -> PSUM
    f32 (accumulated over 2 c-chunks of 128).
  - PSUM -> SBUF bf16 (xb, per-pixel 20x28 window).
  - Dense-band scratch write: one DMA per (stripe, dh) covers all w of that
    pixel row; slicing the window rows p in [dh, dh+13) is uniform within
    the 16-partition dh-group, so scratch holds ONLY the needed 13-row band:
    scr[w*23296 + h*364 + (p-dh)*28 + q]. 728B runs.
  - Band gather: per (h-half, wb) one DMA reads a fully CONTIGUOUS 728KB
    stream (the band layout is sequential in (h, band)) -- full HBM rate.
  - DVE de-shear: tap-row pick + horizontal diagonal q = (w mod 16) + dj
    via strided copy (+ f32 cast) into per-di output tiles.
  - Output write: one DMA per (h-half, di), 512B runs; h-half 0 writes
    overlap h-half 1 gathers.
"""

import numpy as np

C = 256
H = W = 128
KS = 13
KK = 169
RAD = 6
HP = WP = 140  # padded spatial
SA = 8  # block rows (stripe height)
SB = 16  # block cols
NWB = W // SB  # 8 w-blocks per stripe
NST = H // SA  # 16 stripes
WINP = SA + 2 * RAD  # 20 streamed rows per window
WINQ = SB + 2 * RAD  # 28 streamed cols per window
PQ = WINP * WINQ  # 560 elems per pixel in xb
BAND = KS * WINQ  # 364: band elems per pixel (window rows dh..dh+12)
SCR_H = BAND  # 364: scratch stride per h (dense band)
SCR_W = 64 * SCR_H  # 23296: scratch stride per w (per h-half)
HH = H // 2  # 64 rows per h-half

_cache = {}


def _build():
    import concourse.bass as bass
    import concourse.mybir as mybir
    import concourse.tile as tile
    from concourse import bacc

    f32 = mybir.dt.float32
    bf16 = mybir.dt.bfloat16

    nc = bacc.Bacc("TRN2", target_bir_lowering=False, debug=False)
    zt_d = nc.dram_tensor("z_t", [C, H, W], f32, kind="ExternalInput")
    z1_d = nc.dram_tensor("z_t1", [C, H, W], f32, kind="ExternalInput")
    out_d = nc.dram_tensor("out", [KK, H, W], f32, kind="ExternalOutput")

    def alt(i):
        return nc.sync if i % 2 == 0 else nc.scalar

    with tile.TileContext(nc) as tc:
        with tc.tile_pool(name="scrp", bufs=1, space="DRAM") as scrp:
            scr_t = [scrp.tile([W, SCR_W], bf16, tag=f"scr{i}", name=f"scr{i}")
                     for i in range(2)]

            # ================= stripe phase =================
            with (
                tc.tile_pool(name="persist", bufs=1) as pp,
                tc.tile_pool(name="ld", bufs=2) as ldp,
                tc.tile_pool(name="zts", bufs=2) as ztsp,
                tc.tile_pool(name="xbp", bufs=2) as xbp,
                tc.tile_pool(name="psp", bufs=2, space="PSUM") as psp,
            ):
                Z1P = [pp.tile([128, HP * WP], bf16, tag=f"z1p{k}", name=f"z1p{k}")
                       for k in range(2)]
                for k in range(2):
                    nc.vector.memset(Z1P[k][:, :], 0.0)

                zt_tiles = {}

                def emit_loads(s):
                    """Load 32-row slab s of z_t1 and z_t (both c-chunks)."""
                    for k in range(2):
                        z1u = ldp.tile([128, 32 * W], bf16, tag="z1u", name="z1u")
                        src = z1_d.ap()[k * 128:(k + 1) * 128, s * 32:(s + 1) * 32, :]
                        nc.gpsimd.dma_start(
                            z1u.rearrange("c (h w) -> c h w", h=32), src)
                        dst = Z1P[k].rearrange("c (h w) -> c h w", h=HP)[
                            :, RAD + s * 32: RAD + (s + 1) * 32, RAD: RAD + W]
                        nc.vector.tensor_copy(
                            dst, z1u.rearrange("c (h w) -> c h w", h=32))
                    for k in range(2):
                        ztu = ldp.tile([128, 32 * W], bf16, tag="ztu", name="ztu")
                        src = zt_d.ap()[k * 128:(k + 1) * 128, s * 32:(s + 1) * 32, :]
                        nc.gpsimd.dma_start(
                            ztu.rearrange("c (h w) -> c h w", h=32), src)
                        # reorder to block-major with fused 1/16 scale
                        zts = ztsp.tile([128, 4 * 1024], bf16, tag=f"zts{k}",
                                        name=f"zts{k}")
                        for sl in range(4):
                            zt_tiles[(s * 4 + sl, k)] = zts
                            srcv = ztu.rearrange(
                                "c (h wb dw) -> c wb h dw", h=32, wb=NWB)[
                                :, :, sl * SA:(sl + 1) * SA, :]
                            dstv = zts[:, sl * 1024:(sl + 1) * 1024].rearrange(
                                "c (wb dh dw) -> c wb dh dw", wb=NWB, dh=SA)
                            nc.vector.tensor_scalar_mul(dstv, srcv, 1.0 / 16.0)

                def emit_stripe(si):
                    hh, sil = divmod(si, 8)
                    h0 = si * SA
                    xb = xbp.tile([128, NWB * PQ], bf16, tag="xb", name="xb")
                    for wb in range(NWB):
                        w0 = wb * SB
                        ps = [psp.tile([128, 10 * WINQ], f32,
                                       tag=f"ps{i}", name=f"ps{i}")
                              for i in range(2)]
                        for k in range(2):
                            lhsT = zt_tiles[(si, k)][
                                :, (si % 4) * 1024 + wb * 128:
                                   (si % 4) * 1024 + (wb + 1) * 128]
                            for half in range(2):
                                rhs = Z1P[k].rearrange("c (h w) -> c h w", h=HP)[
                                    :, h0 + 10 * half: h0 + 10 * (half + 1),
                                    w0:w0 + WINQ]
                                nc.tensor.matmul(ps[half][:, :], lhsT, rhs,
                                                 start=(k == 0), stop=(k == 1))
                        for half in range(2):
                            dst = xb[:, wb * PQ + half * 10 * WINQ:
                                     wb * PQ + (half + 1) * 10 * WINQ]
                            if wb % 2 == 0:
                                nc.scalar.copy(dst, ps[half][:, :])
                            else:
                                nc.vector.tensor_copy(dst, ps[half][:, :])

                    # dense-band scratch write: one DMA per dh pixel-row;
                    # the p in [dh, dh+13) slice is uniform per dh-group.
                    for dh in range(SA):
                        xsl = xb[dh * SB:(dh + 1) * SB, :]
                        src = bass.AP(xsl.tensor, xsl.offset + dh * WINQ,
                                      [list(xsl.ap[0]), [PQ, NWB], [1, BAND]])
                        dst = bass.AP(scr_t[hh].tensor,
                                      (sil * SA + dh) * SCR_H,
                                      [[SCR_W, SB], [SB * SCR_W, NWB], [1, BAND]])
                        alt(dh).dma_start(dst, src)

                emit_loads(0)
                emit_stripe(0)
                emit_stripe(1)
                emit_loads(1)
                for si in range(2, 6):
                    emit_stripe(si)
                emit_loads(2)
                for si in range(6, 10):
                    emit_stripe(si)
                emit_loads(3)
                for si in range(10, 16):
                    emit_stripe(si)

            # ================= tap phase =================
            with (
                tc.tile_pool(name="bnd", bufs=2) as bndp,
                tc.tile_pool(name="o5p", bufs=1) as o5p,
            ):
                o5 = [o5p.tile([H, KS * W], f32, tag=f"o5_{di}", name=f"o5_{di}")
                      for di in range(KS)]
                for hh in range(2):
                    for wb in range(NWB):
                        par = (hh * NWB + wb) % 2
                        bt = bndp.tile([128, SB * BAND], bf16,
                                       tag="band", name="band")
                        band = bt[par * HH:(par + 1) * HH, :]
                        src = bass.AP(scr_t[hh].tensor, wb * SB * SCR_W,
                                      [[SCR_H, HH], [SCR_W, SB], [1, BAND]])
                        alt(wb).dma_start(
                            band.rearrange("p (w e) -> p w e", e=BAND), src)
                        # de-shear: o5[di][hh*64+p, dj*128 + wb*16 + wh]
                        #   = band[p, wh*364 + di*28 + wh + dj]
                        for di in range(KS):
                            diag = bass.AP(band.tensor, band.offset + di * WINQ,
                                           [list(band.ap[0]),
                                            [BAND + 1, SB], [1, KS]])
                            dst = o5[di][hh * HH:(hh + 1) * HH, :].rearrange(
                                "p (dj w) -> p dj w", dj=KS)[
                                :, :, wb * SB:(wb + 1) * SB].transpose([0, 2, 1])
                            if (wb + di) % 2 == 0:
                                nc.vector.tensor_copy(dst, diag)
                            else:
                                nc.scalar.copy(dst, diag)
                    # output writes for this h-half (overlap next half's
                    # gathers)
                    for di in range(KS):
                        dstw = bass.AP(out_d, di * KS * H * W + hh * HH * W,
                                       [[W, HH], [H * W, KS], [1, W]])
                        alt(di).dma_start(
                            dstw, o5[di][hh * HH:(hh + 1) * HH, :].rearrange(
                                "p (dj w) -> p dj w", dj=KS))

    nc.compile()
    return nc


def _get_nc():
    if "nc" not in _cache:
        _cache["nc"] = _build()
    return _cache["nc"]


def kernel(z_t: np.ndarray, z_t1: np.ndarray) -> np.ndarray:
    from concourse.bass_utils import run_bass_kernel_spmd

    nc = _get_nc()
    z_t = np.ascontiguousarray(z_t, dtype=np.float32)
    z_t1 = np.ascontiguousarray(z_t1, dtype=np.float32)
    B = z_t.shape[0]
    in_maps = [{"z_t": z_t[i], "z_t1": z_t1[i]} for i in range(B)]
    res = run_bass_kernel_spmd(nc, in_maps, core_ids=list(range(B)))
    return np.stack([res.results[i]["out"] for i in range(B)], axis=0)


# revision 14
# speedup vs baseline: 1.1679x; 1.0380x over previous
"""LocalCorrelation (13x13 cost volume) Trainium2 kernel.

Full inputs z_t, z_t1: [8, 256, 128, 128] f32 -> out [8, 169, 128, 128] f32.
out[b, 13*di+dj, h, w] = sum_c z_t[b,c,h,w] * pad(z_t1)[b,c,h+di,w+dj] / 16

Sharding: data-parallel over batch, 1 batch element per NeuronCore (8 cores).

Per-core algorithm (SPMD, identical program):
  - Slab-staged input load (f32->bf16 SWDGE cast) interleaved with compute;
    1/sqrt(C) scale fused into the z_t reorder copy; z_t stationary tiles
    streamed per-slab (rolling pool).
  - Per 8x16 output-pixel block: TensorE "block gram" matmuls, stationary =
    z_t block [c,128 pixels], streaming = padded z_t1 20x28 window -> PSUM
    f32 (accumulated over 2 c-chunks of 128).
  - PSUM -> SBUF bf16 (xb, per-pixel 20x28 window).
  - Dense-band scratch write: one DMA per (stripe, dh) covers all w of that
    pixel row; slicing the window rows p in [dh, dh+13) is uniform within
    the 16-partition dh-group, so scratch holds ONLY the needed 13-row band:
    scr[w*23296 + h*364 + (p-dh)*28 + q]. 728B runs.
  - Band gather: per (h-half, wb) one DMA reads a fully CONTIGUOUS 728KB
    stream (the band layout is sequential in (h, band)) -- full HBM rate.
  - DVE de-shear: tap-row pick + horizontal diagonal q = (w mod 16) + dj
    via strided copy (+ f32 cast) into per-di output tiles.
  - Output write: one DMA per (h-half, di), 512B runs; h-half 0 writes
    overlap h-half 1 gathers.
"""

import numpy as np

C = 256
H = W = 128
KS = 13
KK = 169
RAD = 6
HP = WP = 140  # padded spatial
SA = 8  # block rows (stripe height)
SB = 16  # block cols
NWB = W // SB  # 8 w-blocks per stripe
NST = H // SA  # 16 stripes
WINP = SA + 2 * RAD  # 20 streamed rows per window
WINQ = SB + 2 * RAD  # 28 streamed cols per window
PQ = WINP * WINQ  # 560 elems per pixel in xb
BAND = KS * WINQ  # 364: band elems per pixel (window rows dh..dh+12)
SCR_DW = SA * BAND  # 2912: scratch stride per dw (w mod 16)
SCR_HM = W * BAND  # 46592: scratch stride per h row (h-major layout)
HH = H // 2  # 64 rows per h-half
NDWG = 4  # dw-groups of 4 for the gather
GW = 4 * SCR_DW  # 11648: gathered elems per (h, dw-group)

_cache = {}


def _build():
    import concourse.bass as bass
    import concourse.mybir as mybir
    import concourse.tile as tile
    from concourse import bacc

    f32 = mybir.dt.float32
    bf16 = mybir.dt.bfloat16

    nc = bacc.Bacc("TRN2", target_bir_lowering=False, debug=False)
    zt_d = nc.dram_tensor("z_t", [C, H, W], f32, kind="ExternalInput")
    z1_d = nc.dram_tensor("z_t1", [C, H, W], f32, kind="ExternalInput")
    out_d = nc.dram_tensor("out", [KK, H, W], f32, kind="ExternalOutput")

    def alt(i):
        return nc.sync if i % 2 == 0 else nc.scalar

    with tile.TileContext(nc) as tc:
        with tc.tile_pool(name="scrp", bufs=1, space="DRAM") as scrp:
            scr_t = [scrp.tile([HH, SCR_HM], bf16, tag=f"scr{i}", name=f"scr{i}")
                     for i in range(2)]

            # ================= stripe phase =================
            with (
                tc.tile_pool(name="persist", bufs=1) as pp,
                tc.tile_pool(name="ld", bufs=2) as ldp,
                tc.tile_pool(name="zts", bufs=2) as ztsp,
                tc.tile_pool(name="xbp", bufs=2) as xbp,
                tc.tile_pool(name="psp", bufs=2, space="PSUM") as psp,
            ):
                Z1P = [pp.tile([128, HP * WP], bf16, tag=f"z1p{k}", name=f"z1p{k}")
                       for k in range(2)]
                for k in range(2):
                    nc.vector.memset(Z1P[k][:, :], 0.0)

                zt_tiles = {}

                def emit_loads(s):
                    """Load 32-row slab s of z_t1 and z_t (both c-chunks)."""
                    for k in range(2):
                        z1u = ldp.tile([128, 32 * W], bf16, tag="z1u", name="z1u")
                        src = z1_d.ap()[k * 128:(k + 1) * 128, s * 32:(s + 1) * 32, :]
                        nc.gpsimd.dma_start(
                            z1u.rearrange("c (h w) -> c h w", h=32), src)
                        dst = Z1P[k].rearrange("c (h w) -> c h w", h=HP)[
                            :, RAD + s * 32: RAD + (s + 1) * 32, RAD: RAD + W]
                        nc.vector.tensor_copy(
                            dst, z1u.rearrange("c (h w) -> c h w", h=32))
                    for k in range(2):
                        ztu = ldp.tile([128, 32 * W], bf16, tag="ztu", name="ztu")
                        src = zt_d.ap()[k * 128:(k + 1) * 128, s * 32:(s + 1) * 32, :]
                        nc.gpsimd.dma_start(
                            ztu.rearrange("c (h w) -> c h w", h=32), src)
                        # reorder to block-major with fused 1/16 scale
                        zts = ztsp.tile([128, 4 * 1024], bf16, tag=f"zts{k}",
                                        name=f"zts{k}")
                        for sl in range(4):
                            zt_tiles[(s * 4 + sl, k)] = zts
                            srcv = ztu.rearrange(
                                "c (h wb dw) -> c wb h dw", h=32, wb=NWB)[
                                :, :, sl * SA:(sl + 1) * SA, :]
                            dstv = zts[:, sl * 1024:(sl + 1) * 1024].rearrange(
                                "c (wb dh dw) -> c wb dh dw", wb=NWB, dh=SA)
                            nc.vector.tensor_scalar_mul(dstv, srcv, 1.0 / 16.0)

                def emit_stripe(si):
                    hh, sil = divmod(si, 8)
                    h0 = si * SA
                    xb = xbp.tile([128, NWB * PQ], bf16, tag="xb", name="xb")
                    for wb in range(NWB):
                        w0 = wb * SB
                        ps = [psp.tile([128, 10 * WINQ], f32,
                                       tag=f"ps{i}", name=f"ps{i}")
                              for i in range(2)]
                        for k in range(2):
                            lhsT = zt_tiles[(si, k)][
                                :, (si % 4) * 1024 + wb * 128:
                                   (si % 4) * 1024 + (wb + 1) * 128]
                            for half in range(2):
                                rhs = Z1P[k].rearrange("c (h w) -> c h w", h=HP)[
                                    :, h0 + 10 * half: h0 + 10 * (half + 1),
                                    w0:w0 + WINQ]
                                nc.tensor.matmul(ps[half][:, :], lhsT, rhs,
                                                 start=(k == 0), stop=(k == 1))
                        for half in range(2):
                            dst = xb[:, wb * PQ + half * 10 * WINQ:
                                     wb * PQ + (half + 1) * 10 * WINQ]
                            if wb % 2 == 0:
                                nc.scalar.copy(dst, ps[half][:, :])
                            else:
                                nc.vector.tensor_copy(dst, ps[half][:, :])

                    # dense-band scratch write: one DMA per dh pixel-row;
                    # the p in [dh, dh+13) slice is uniform per dh-group.
                    # h-major scratch: addr = h*46592 + dw*2912 + wb*364 + e,
                    # fully contiguous per partition -> 5.8KB descriptors.
                    for dh in range(SA):
                        xsl = xb[dh * SB:(dh + 1) * SB, :]
                        src = bass.AP(xsl.tensor, xsl.offset + dh * WINQ,
                                      [list(xsl.ap[0]), [PQ, NWB], [1, BAND]])
                        dst = bass.AP(scr_t[hh].tensor,
                                      (sil * SA + dh) * SCR_HM,
                                      [[SCR_DW, SB], [BAND, NWB], [1, BAND]])
                        alt(dh).dma_start(dst, src)

                emit_loads(0)
                emit_stripe(0)
                emit_stripe(1)
                emit_loads(1)
                for si in range(2, 6):
                    emit_stripe(si)
                emit_loads(2)
                for si in range(6, 10):
                    emit_stripe(si)
                emit_loads(3)
                for si in range(10, 16):
                    emit_stripe(si)

            # ================= tap phase =================
            with (
                tc.tile_pool(name="bnd", bufs=2) as bndp,
                tc.tile_pool(name="o5p", bufs=1) as o5p,
            ):
                o5 = [o5p.tile([H, KS * W], f32, tag=f"o5_{di}", name=f"o5_{di}")
                      for di in range(KS)]
                for hh in range(2):
                    for dwg in range(NDWG):
                        par = (hh * NDWG + dwg) % 2
                        bt = bndp.tile([128, GW], bf16, tag="band", name="band")
                        band = bt[par * HH:(par + 1) * HH, :]
                        src = bass.AP(scr_t[hh].tensor, dwg * GW,
                                      [[SCR_HM, HH], [1, GW]])
                        alt(dwg).dma_start(band, src)
                        # de-shear: o5[di][hh*64+p, dj*128 + wb*16 + dwg*4+dwl]
                        #   = band[p, dwl*2912 + wb*364 + di*28 + dwg*4+dwl + dj]
                        for di in range(KS):
                            diag = bass.AP(band.tensor,
                                           band.offset + di * WINQ + dwg * NDWG,
                                           [list(band.ap[0]), [SCR_DW + 1, 4],
                                            [BAND, NWB], [1, KS]])
                            o5sl = o5[di][hh * HH:(hh + 1) * HH, :]
                            dst = bass.AP(o5sl.tensor,
                                          o5sl.offset + dwg * NDWG,
                                          [list(o5sl.ap[0]), [1, 4],
                                           [SB, NWB], [W, KS]])
                            if (dwg + di) % 2 == 0:
                                nc.vector.tensor_copy(dst, diag)
                            else:
                                nc.scalar.copy(dst, diag)
                    # output writes for this h-half (overlap next half's
                    # gathers)
                    for di in range(KS):
                        dstw = bass.AP(out_d, di * KS * H * W + hh * HH * W,
                                       [[W, HH], [H * W, KS], [1, W]])
                        alt(di).dma_start(
                            dstw, o5[di][hh * HH:(hh + 1) * HH, :].rearrange(
                                "p (dj w) -> p dj w", dj=KS))

    nc.compile()
    return nc


def _get_nc():
    if "nc" not in _cache:
        _cache["nc"] = _build()
    return _cache["nc"]


def kernel(z_t: np.ndarray, z_t1: np.ndarray) -> np.ndarray:
    from concourse.bass_utils import run_bass_kernel_spmd

    nc = _get_nc()
    z_t = np.ascontiguousarray(z_t, dtype=np.float32)
    z_t1 = np.ascontiguousarray(z_t1, dtype=np.float32)
    B = z_t.shape[0]
    in_maps = [{"z_t": z_t[i], "z_t1": z_t1[i]} for i in range(B)]
    res = run_bass_kernel_spmd(nc, in_maps, core_ids=list(range(B)))
    return np.stack([res.results[i]["out"] for i in range(B)], axis=0)


# revision 15
# speedup vs baseline: 1.2015x; 1.0287x over previous
"""LocalCorrelation (13x13 cost volume) Trainium2 kernel.

Full inputs z_t, z_t1: [8, 256, 128, 128] f32 -> out [8, 169, 128, 128] f32.
out[b, 13*di+dj, h, w] = sum_c z_t[b,c,h,w] * pad(z_t1)[b,c,h+di,w+dj] / 16

Sharding: data-parallel over batch, 1 batch element per NeuronCore (8 cores).

Per-core algorithm (SPMD, identical program):
  - Slab-staged input load (f32->bf16 SWDGE cast) interleaved with compute;
    1/sqrt(C) scale fused into the z_t reorder copy; z_t stationary tiles
    streamed per-slab (rolling pool).
  - Per 8x16 output-pixel block: TensorE "block gram" matmuls, stationary =
    z_t block [c,128 pixels], streaming = padded z_t1 20x28 window -> PSUM
    f32 (accumulated over 2 c-chunks of 128).
  - PSUM -> SBUF bf16 (xb, per-pixel 20x28 window).
  - Dense-band scratch write: one DMA per (stripe, dh) covers all w of that
    pixel row; slicing the window rows p in [dh, dh+13) is uniform within
    the 16-partition dh-group, so scratch holds ONLY the needed 13-row band:
    scr[w*23296 + h*364 + (p-dh)*28 + q]. 728B runs.
  - Band gather: per (h-half, wb) one DMA reads a fully CONTIGUOUS 728KB
    stream (the band layout is sequential in (h, band)) -- full HBM rate.
  - DVE de-shear: tap-row pick + horizontal diagonal q = (w mod 16) + dj
    via strided copy (+ f32 cast) into per-di output tiles.
  - Output write: one DMA per (h-half, di), 512B runs; h-half 0 writes
    overlap h-half 1 gathers.
"""

import numpy as np

C = 256
H = W = 128
KS = 13
KK = 169
RAD = 6
HP = WP = 140  # padded spatial
SA = 8  # block rows (stripe height)
SB = 16  # block cols
NWB = W // SB  # 8 w-blocks per stripe
NST = H // SA  # 16 stripes
WINP = SA + 2 * RAD  # 20 streamed rows per window
WINQ = SB + 2 * RAD  # 28 streamed cols per window
PQ = WINP * WINQ  # 560 elems per pixel in xb
BAND = KS * WINQ  # 364: band elems per pixel (window rows dh..dh+12)
SCR_DW = SA * BAND  # 2912: scratch stride per dw (w mod 16)
SCR_HM = W * BAND  # 46592: scratch stride per h row (h-major layout)
HH = H // 2  # 64 rows per h-half
NDWG = 4  # dw-groups of 4 for the gather
GW = 4 * SCR_DW  # 11648: gathered elems per (h, dw-group)

_cache = {}


def _build():
    import concourse.bass as bass
    import concourse.mybir as mybir
    import concourse.tile as tile
    from concourse import bacc

    f32 = mybir.dt.float32
    bf16 = mybir.dt.bfloat16

    nc = bacc.Bacc("TRN2", target_bir_lowering=False, debug=False)
    zt_d = nc.dram_tensor("z_t", [C, H, W], f32, kind="ExternalInput")
    z1_d = nc.dram_tensor("z_t1", [C, H, W], f32, kind="ExternalInput")
    out_d = nc.dram_tensor("out", [KK, H, W], f32, kind="ExternalOutput")

    def alt(i):
        return nc.sync if i % 2 == 0 else nc.scalar

    with tile.TileContext(nc) as tc:
        with tc.tile_pool(name="scrp", bufs=1, space="DRAM") as scrp:
            scr_t = [scrp.tile([HH, SCR_HM], bf16, tag=f"scr{i}", name=f"scr{i}")
                     for i in range(2)]

            # ================= stripe phase =================
            with (
                tc.tile_pool(name="persist", bufs=1) as pp,
                tc.tile_pool(name="ld", bufs=2) as ldp,
                tc.tile_pool(name="zts", bufs=2) as ztsp,
                tc.tile_pool(name="xbp", bufs=2) as xbp,
                tc.tile_pool(name="psp", bufs=4, space="PSUM") as psp,
            ):
                Z1P = [pp.tile([128, HP * WP], bf16, tag=f"z1p{k}", name=f"z1p{k}")
                       for k in range(2)]
                for k in range(2):
                    # zero only the 6-wide pad frame, not the whole tile
                    zv = Z1P[k].rearrange("c (h w) -> c h w", h=HP)
                    nc.vector.memset(zv[:, 0:RAD, :], 0.0)
                    nc.vector.memset(zv[:, HP - RAD:HP, :], 0.0)
                    nc.vector.memset(zv[:, RAD:HP - RAD, 0:RAD], 0.0)
                    nc.vector.memset(zv[:, RAD:HP - RAD, WP - RAD:WP], 0.0)

                zt_tiles = {}

                def emit_loads(s):
                    """Load 32-row slab s of z_t1 and z_t (both c-chunks)."""
                    z1u, ztu = [], []
                    for k in range(2):
                        u = ldp.tile([128, 32 * W], bf16, tag="z1u", name="z1u")
                        z1u.append(u)
                        src = z1_d.ap()[k * 128:(k + 1) * 128, s * 32:(s + 1) * 32, :]
                        nc.gpsimd.dma_start(
                            u.rearrange("c (h w) -> c h w", h=32), src)
                    for k in range(2):
                        u = ldp.tile([128, 32 * W], bf16, tag="ztu", name="ztu")
                        ztu.append(u)
                        src = zt_d.ap()[k * 128:(k + 1) * 128, s * 32:(s + 1) * 32, :]
                        nc.gpsimd.dma_start(
                            u.rearrange("c (h w) -> c h w", h=32), src)
                    for k in range(2):
                        dst = Z1P[k].rearrange("c (h w) -> c h w", h=HP)[
                            :, RAD + s * 32: RAD + (s + 1) * 32, RAD: RAD + W]
                        nc.gpsimd.tensor_copy(
                            dst, z1u[k].rearrange("c (h w) -> c h w", h=32))
                    for k in range(2):
                        # reorder to block-major with fused 1/16 scale
                        zts = ztsp.tile([128, 4 * 1024], bf16, tag=f"zts{k}",
                                        name=f"zts{k}")
                        for sl in range(4):
                            zt_tiles[(s * 4 + sl, k)] = zts
                            srcv = ztu[k].rearrange(
                                "c (h wb dw) -> c wb h dw", h=32, wb=NWB)[
                                :, :, sl * SA:(sl + 1) * SA, :]
                            dstv = zts[:, sl * 1024:(sl + 1) * 1024].rearrange(
                                "c (wb dh dw) -> c wb dh dw", wb=NWB, dh=SA)
                            nc.vector.tensor_scalar_mul(dstv, srcv, 1.0 / 16.0)

                def emit_stripe(si):
                    hh, sil = divmod(si, 8)
                    h0 = si * SA
                    xb = xbp.tile([128, NWB * PQ], bf16, tag="xb", name="xb")
                    for wb in range(NWB):
                        w0 = wb * SB
                        ps = [psp.tile([128, 10 * WINQ], f32,
                                       tag=f"ps{i}", name=f"ps{i}")
                              for i in range(2)]
                        for k in range(2):
                            lhsT = zt_tiles[(si, k)][
                                :, (si % 4) * 1024 + wb * 128:
                                   (si % 4) * 1024 + (wb + 1) * 128]
                            for half in range(2):
                                rhs = Z1P[k].rearrange("c (h w) -> c h w", h=HP)[
                                    :, h0 + 10 * half: h0 + 10 * (half + 1),
                                    w0:w0 + WINQ]
                                nc.tensor.matmul(ps[half][:, :], lhsT, rhs,
                                                 start=(k == 0), stop=(k == 1))
                        for half in range(2):
                            dst = xb[:, wb * PQ + half * 10 * WINQ:
                                     wb * PQ + (half + 1) * 10 * WINQ]
                            if wb % 2 == 0:
                                nc.scalar.copy(dst, ps[half][:, :])
                            else:
                                nc.vector.tensor_copy(dst, ps[half][:, :])

                    # dense-band scratch write: one DMA per dh pixel-row;
                    # the p in [dh, dh+13) slice is uniform per dh-group.
                    # h-major scratch: addr = h*46592 + dw*2912 + wb*364 + e,
                    # fully contiguous per partition -> 5.8KB descriptors.
                    for dh in range(SA):
                        xsl = xb[dh * SB:(dh + 1) * SB, :]
                        src = bass.AP(xsl.tensor, xsl.offset + dh * WINQ,
                                      [list(xsl.ap[0]), [PQ, NWB], [1, BAND]])
                        dst = bass.AP(scr_t[hh].tensor,
                                      (sil * SA + dh) * SCR_HM,
                                      [[SCR_DW, SB], [BAND, NWB], [1, BAND]])
                        alt(dh).dma_start(dst, src)

                emit_loads(0)
                emit_loads(1)
                for si in range(0, 4):
                    emit_stripe(si)
                emit_loads(2)
                for si in range(4, 10):
                    emit_stripe(si)
                emit_loads(3)
                for si in range(10, 16):
                    emit_stripe(si)

            # ================= tap phase =================
            with (
                tc.tile_pool(name="bnd", bufs=2) as bndp,
                tc.tile_pool(name="o5p", bufs=1) as o5p,
            ):
                o5 = [o5p.tile([H, KS * W], f32, tag=f"o5_{di}", name=f"o5_{di}")
                      for di in range(KS)]
                for hh in range(2):
                    for dwg in range(NDWG):
                        par = (hh * NDWG + dwg) % 2
                        bt = bndp.tile([128, GW], bf16, tag="band", name="band")
                        band = bt[par * HH:(par + 1) * HH, :]
                        src = bass.AP(scr_t[hh].tensor, dwg * GW,
                                      [[SCR_HM, HH], [1, GW]])
                        alt(dwg).dma_start(band, src)
                        # de-shear: o5[di][hh*64+p, dj*128 + wb*16 + dwg*4+dwl]
                        #   = band[p, dwl*2912 + wb*364 + di*28 + dwg*4+dwl + dj]
                        for di in range(KS):
                            diag = bass.AP(band.tensor,
                                           band.offset + di * WINQ + dwg * NDWG,
                                           [list(band.ap[0]), [SCR_DW + 1, 4],
                                            [BAND, NWB], [1, KS]])
                            o5sl = o5[di][hh * HH:(hh + 1) * HH, :]
                            dst = bass.AP(o5sl.tensor,
                                          o5sl.offset + dwg * NDWG,
                                          [list(o5sl.ap[0]), [1, 4],
                                           [SB, NWB], [W, KS]])
                            if (dwg + di) % 2 == 0:
                                nc.vector.tensor_copy(dst, diag)
                            else:
                                nc.scalar.copy(dst, diag)
                    # output writes for this h-half (overlap next half's
                    # gathers)
                    for di in range(KS):
                        dstw = bass.AP(out_d, di * KS * H * W + hh * HH * W,
                                       [[W, HH], [H * W, KS], [1, W]])
                        alt(di).dma_start(
                            dstw, o5[di][hh * HH:(hh + 1) * HH, :].rearrange(
                                "p (dj w) -> p dj w", dj=KS))

    nc.compile()
    return nc


def _get_nc():
    if "nc" not in _cache:
        _cache["nc"] = _build()
    return _cache["nc"]


def kernel(z_t: np.ndarray, z_t1: np.ndarray) -> np.ndarray:
    from concourse.bass_utils import run_bass_kernel_spmd

    nc = _get_nc()
    z_t = np.ascontiguousarray(z_t, dtype=np.float32)
    z_t1 = np.ascontiguousarray(z_t1, dtype=np.float32)
    B = z_t.shape[0]
    in_maps = [{"z_t": z_t[i], "z_t1": z_t1[i]} for i in range(B)]
    res = run_bass_kernel_spmd(nc, in_maps, core_ids=list(range(B)))
    return np.stack([res.results[i]["out"] for i in range(B)], axis=0)


# revision 17
# speedup vs baseline: 1.2891x; 1.0730x over previous
"""LocalCorrelation (13x13 cost volume) Trainium2 kernel.

Full inputs z_t, z_t1: [8, 256, 128, 128] f32 -> out [8, 169, 128, 128] f32.
out[b, 13*di+dj, h, w] = sum_c z_t[b,c,h,w] * pad(z_t1)[b,c,h+di,w+dj] / 16

Sharding: data-parallel over batch, 1 batch element per NeuronCore (8 cores).

Per-core algorithm (SPMD, identical program):
  - Slab-staged input load (f32->bf16 SWDGE cast) interleaved with compute;
    1/sqrt(C) scale fused into the z_t reorder copy; z_t stationary tiles
    streamed per-slab (rolling pool).
  - Per 8x16 output-pixel block: TensorE "block gram" matmuls, stationary =
    z_t block [c,128 pixels], streaming = padded z_t1 20x28 window -> PSUM
    f32 (accumulated over 2 c-chunks of 128).
  - PSUM -> SBUF bf16 (xb, per-pixel 20x28 window).
  - Dense-band scratch write: one DMA per (stripe, dh) covers all w of that
    pixel row; slicing the window rows p in [dh, dh+13) is uniform within
    the 16-partition dh-group, so scratch holds ONLY the needed 13-row band:
    scr[w*23296 + h*364 + (p-dh)*28 + q]. 728B runs.
  - Band gather: per (h-half, wb) one DMA reads a fully CONTIGUOUS 728KB
    stream (the band layout is sequential in (h, band)) -- full HBM rate.
  - DVE de-shear: tap-row pick + horizontal diagonal q = (w mod 16) + dj
    via strided copy (+ f32 cast) into per-di output tiles.
  - Output write: one DMA per (h-half, di), 512B runs; h-half 0 writes
    overlap h-half 1 gathers.
"""

import numpy as np

C = 256
H = W = 128
KS = 13
KK = 169
RAD = 6
HP = WP = 140  # padded spatial
SA = 8  # block rows (stripe height)
SB = 16  # block cols
NWB = W // SB  # 8 w-blocks per stripe
NST = H // SA  # 16 stripes
WINP = SA + 2 * RAD  # 20 streamed rows per window
WINQ = SB + 2 * RAD  # 28 streamed cols per window
PQ = WINP * WINQ  # 560 elems per pixel in xb
BAND = KS * WINQ  # 364: band elems per pixel (window rows dh..dh+12)
SCR_DW = SA * BAND  # 2912: scratch stride per dw (w mod 16)
SCR_HM = W * BAND  # 46592: scratch stride per h row (h-major layout)
HH = H // 2  # 64 rows per h-half
NDWG = 4  # dw-groups of 4 for the gather
GW = 4 * SCR_DW  # 11648: gathered elems per (h, dw-group)

_cache = {}


def _build():
    import concourse.bass as bass
    import concourse.mybir as mybir
    import concourse.tile as tile
    from concourse import bacc

    f32 = mybir.dt.float32
    bf16 = mybir.dt.bfloat16

    nc = bacc.Bacc("TRN2", target_bir_lowering=False, debug=False)
    zt_d = nc.dram_tensor("z_t", [C, H, W], f32, kind="ExternalInput")
    z1_d = nc.dram_tensor("z_t1", [C, H, W], f32, kind="ExternalInput")
    out_d = nc.dram_tensor("out", [KK, H, W], f32, kind="ExternalOutput")

    def alt(i):
        return nc.sync if i % 2 == 0 else nc.scalar

    with tile.TileContext(nc) as tc:
        with tc.tile_pool(name="scrp", bufs=1, space="DRAM") as scrp:
            scr_t = [scrp.tile([HH, SCR_HM], bf16, tag=f"scr{i}", name=f"scr{i}")
                     for i in range(2)]

            # ================= stripe phase =================
            with (
                tc.tile_pool(name="persist", bufs=1) as pp,
                tc.tile_pool(name="zts", bufs=4) as ztsp,
                tc.tile_pool(name="ztb", bufs=4) as ztbp,
                tc.tile_pool(name="xbp", bufs=2) as xbp,
                tc.tile_pool(name="psp", bufs=4, space="PSUM") as psp,
            ):
                Z1P = [pp.tile([128, HP * WP], bf16, tag=f"z1p{k}", name=f"z1p{k}")
                       for k in range(2)]
                for k in range(2):
                    # zero only the 6-wide pad frame, not the whole tile
                    zv = Z1P[k].rearrange("c (h w) -> c h w", h=HP)
                    nc.vector.memset(zv[:, 0:RAD, :], 0.0)
                    nc.vector.memset(zv[:, HP - RAD:HP, :], 0.0)
                    nc.vector.memset(zv[:, RAD:HP - RAD, 0:RAD], 0.0)
                    nc.vector.memset(zv[:, RAD:HP - RAD, WP - RAD:WP], 0.0)

                zt_tiles = {}

                def emit_loads(s):
                    """Load 32-row slab s of z_t1 and z_t (both c-chunks).
                    Direct SWDGE cast-DMAs: z_t1 lands in the padded Z1P
                    interior; z_t lands contiguous (matmul reads it with a
                    strided stationary AP -- no reorder copy)."""
                    for k in range(2):
                        dst = Z1P[k].rearrange("c (h w) -> c h w", h=HP)[
                            :, RAD + s * 32: RAD + (s + 1) * 32, RAD: RAD + W]
                        src = z1_d.ap()[k * 128:(k + 1) * 128, s * 32:(s + 1) * 32, :]
                        nc.gpsimd.dma_start(dst, src)
                    for k in range(2):
                        zts = ztsp.tile([128, 4 * 1024], bf16, tag=f"zts{k}",
                                        name=f"zts{k}")
                        for sl in range(4):
                            zt_tiles[(s * 4 + sl, k)] = zts
                        src = zt_d.ap()[k * 128:(k + 1) * 128, s * 32:(s + 1) * 32, :]
                        nc.gpsimd.dma_start(
                            zts.rearrange("c (h w) -> c h w", h=32), src)

                def emit_stripe(si):
                    hh, sil = divmod(si, 8)
                    h0 = si * SA
                    # block-major stationary for this stripe (GpSimd copy)
                    ztb = {}
                    for k in range(2):
                        t = ztbp.tile([128, SA * W], bf16, tag=f"ztb{k}",
                                      name=f"ztb{k}")
                        ztb[k] = t
                        srcv = zt_tiles[(si, k)].rearrange(
                            "c (h wb dw) -> c wb h dw", h=32, wb=NWB)[
                            :, :, (si % 4) * SA:(si % 4 + 1) * SA, :]
                        dstv = t.rearrange("c (wb dh dw) -> c wb dh dw",
                                           wb=NWB, dh=SA)
                        nc.gpsimd.tensor_copy(dstv, srcv)
                    xb = xbp.tile([128, NWB * PQ], bf16, tag="xb", name="xb")
                    for wb in range(NWB):
                        w0 = wb * SB
                        ps = [psp.tile([128, 10 * WINQ], f32,
                                       tag=f"ps{i}", name=f"ps{i}")
                              for i in range(2)]
                        for k in range(2):
                            lhsT = ztb[k][:, wb * 128:(wb + 1) * 128]
                            for half in range(2):
                                rhs = Z1P[k].rearrange("c (h w) -> c h w", h=HP)[
                                    :, h0 + 10 * half: h0 + 10 * (half + 1),
                                    w0:w0 + WINQ]
                                nc.tensor.matmul(ps[half][:, :], lhsT, rhs,
                                                 start=(k == 0), stop=(k == 1))
                        for half in range(2):
                            dst = xb[:, wb * PQ + half * 10 * WINQ:
                                     wb * PQ + (half + 1) * 10 * WINQ]
                            if wb % 2 == 0:
                                nc.scalar.mul(dst, ps[half][:, :], 1.0 / 16.0)
                            else:
                                nc.vector.tensor_scalar_mul(
                                    dst, ps[half][:, :], 1.0 / 16.0)

                    # dense-band scratch write: one DMA per dh pixel-row;
                    # the p in [dh, dh+13) slice is uniform per dh-group.
                    # h-major scratch: addr = h*46592 + dw*2912 + wb*364 + e,
                    # fully contiguous per partition -> 5.8KB descriptors.
                    for dh in range(SA):
                        xsl = xb[dh * SB:(dh + 1) * SB, :]
                        src = bass.AP(xsl.tensor, xsl.offset + dh * WINQ,
                                      [list(xsl.ap[0]), [PQ, NWB], [1, BAND]])
                        dst = bass.AP(scr_t[hh].tensor,
                                      (sil * SA + dh) * SCR_HM,
                                      [[SCR_DW, SB], [BAND, NWB], [1, BAND]])
                        alt(dh).dma_start(dst, src)

                for s in range(4):
                    emit_loads(s)
                for si in range(NST):
                    emit_stripe(si)

            # ================= tap phase =================
            with (
                tc.tile_pool(name="bnd", bufs=2) as bndp,
                tc.tile_pool(name="o5p", bufs=1) as o5p,
            ):
                o5 = [o5p.tile([H, KS * W], f32, tag=f"o5_{di}", name=f"o5_{di}")
                      for di in range(KS)]
                for hh in range(2):
                    for dwg in range(NDWG):
                        par = (hh * NDWG + dwg) % 2
                        bt = bndp.tile([128, GW], bf16, tag="band", name="band")
                        band = bt[par * HH:(par + 1) * HH, :]
                        src = bass.AP(scr_t[hh].tensor, dwg * GW,
                                      [[SCR_HM, HH], [1, GW]])
                        alt(dwg).dma_start(band, src)
                        # de-shear: o5[di][hh*64+p, dj*128 + wb*16 + dwg*4+dwl]
                        #   = band[p, dwl*2912 + wb*364 + di*28 + dwg*4+dwl + dj]
                        for di in range(KS):
                            diag = bass.AP(band.tensor,
                                           band.offset + di * WINQ + dwg * NDWG,
                                           [list(band.ap[0]), [SCR_DW + 1, 4],
                                            [BAND, NWB], [1, KS]])
                            o5sl = o5[di][hh * HH:(hh + 1) * HH, :]
                            dst = bass.AP(o5sl.tensor,
                                          o5sl.offset + dwg * NDWG,
                                          [list(o5sl.ap[0]), [1, 4],
                                           [SB, NWB], [W, KS]])
                            if (dwg + di) % 2 == 0:
                                nc.vector.tensor_copy(dst, diag)
                            else:
                                nc.scalar.copy(dst, diag)
                    # output writes for this h-half (overlap next half's
                    # gathers)
                    for di in range(KS):
                        dstw = bass.AP(out_d, di * KS * H * W + hh * HH * W,
                                       [[W, HH], [H * W, KS], [1, W]])
                        alt(di).dma_start(
                            dstw, o5[di][hh * HH:(hh + 1) * HH, :].rearrange(
                                "p (dj w) -> p dj w", dj=KS))

    nc.compile()
    return nc


def _get_nc():
    if "nc" not in _cache:
        _cache["nc"] = _build()
    return _cache["nc"]


def kernel(z_t: np.ndarray, z_t1: np.ndarray) -> np.ndarray:
    from concourse.bass_utils import run_bass_kernel_spmd

    nc = _get_nc()
    z_t = np.ascontiguousarray(z_t, dtype=np.float32)
    z_t1 = np.ascontiguousarray(z_t1, dtype=np.float32)
    B = z_t.shape[0]
    in_maps = [{"z_t": z_t[i], "z_t1": z_t1[i]} for i in range(B)]
    res = run_bass_kernel_spmd(nc, in_maps, core_ids=list(range(B)))
    return np.stack([res.results[i]["out"] for i in range(B)], axis=0)
